# revision 4
# baseline (speedup 1.0000x reference)
"""Trainium2 Bass kernel for nn_CustomResidualAttentionBlock (open_clip-style block).

Sharding: sequence-parallel over 8 cores. Core c owns 512 tokens
(b = c // 4, tokens [512*(c%4) : 512*(c%4+1)]). Each core computes q/k/v for its
own tokens, l2-normalizes k (and q) locally via a ones-block matmul trick, then
one AllGather per 4-core batch group distributes (kT, v) for the full 2048-key
sequence. Attention, out-proj, residuals and the MLP are fully local.

Host-side folds (exact math, done in fp32 on host):
  - ln1_g into wqkT/wvT columns; ln1_b @ W^T into the qkv biases
  - ln2_g into wfcT; ln2_b @ fc_w^T into fc bias
  - ls1 into ln_attn affine (g' = ls1*g, b' = ls1*b)
  - ls2 into proj weights/bias
  - logit_scale (clamped+exp'd) into the q-norm ones-block (1/lsc^2 entries)
  - head_scale into the rowsum-replication lhsT (1/hs entries)

All big matmuls run in bf16 with fp32 PSUM accumulation; layernorm statistics,
softmax row sums and normalization factors stay in fp32.
"""
import numpy as np
import ml_dtypes

import concourse.bass as bass
import concourse.mybir as mybir
import concourse.tile as tile
from concourse import bacc
from concourse.bass_utils import run_bass_kernel_spmd
from concourse.masks import make_identity

F32 = mybir.dt.float32
BF16 = mybir.dt.bfloat16
BF_NP = ml_dtypes.bfloat16
AF = mybir.ActivationFunctionType
ALU = mybir.AluOpType

B, L, C, H = 2, 2048, 1024, 16
HD = C // H          # 64
MLP = 4 * C          # 4096
N_CORES = 8
RANKS = 4            # cores per batch group
T = (B * L) // N_CORES  # 512 own tokens per core
TT = T // 128        # 4 token tiles
CT = C // 128        # 8 channel tiles
HP = H // 2          # 8 head pairs
KM = L // 128        # 16 key chunks
MT_FC = MLP // 128   # 32
LN_EPS = 1e-5

# exec time of the last launch (ns), populated when TRACE is set
TRACE = False
TRACE_DIR = "/tmp/bass_trace"
LAST_EXEC_NS = None
LAST_RESULTS = None

_NC_CACHE = None


def _build():
    nc = bacc.Bacc(None, target_bir_lowering=False, debug=False, num_devices=N_CORES)

    # ---- I/O ----
    x_d = nc.dram_tensor("x", [T, C], F32, kind="ExternalInput")
    out_d = nc.dram_tensor("out", [T, C], F32, kind="ExternalOutput")
    wqk_d = nc.dram_tensor("wqkT", [C, 2 * C], BF16, kind="ExternalInput")
    wv_d = nc.dram_tensor("wvT", [C, C], BF16, kind="ExternalInput")
    wo_d = nc.dram_tensor("woT", [C, C], BF16, kind="ExternalInput")
    wfc_d = nc.dram_tensor("wfcT", [C, MLP], BF16, kind="ExternalInput")
    wpj_d = nc.dram_tensor("wprojT", [MLP, C], BF16, kind="ExternalInput")
    qkb_d = nc.dram_tensor("qkb", [128, 16], F32, kind="ExternalInput")
    vb_d = nc.dram_tensor("vb", [1, C], F32, kind="ExternalInput")
    outb_d = nc.dram_tensor("outb", [1, C], F32, kind="ExternalInput")
    fcb_d = nc.dram_tensor("fcb", [128, 32], F32, kind="ExternalInput")
    pjb_d = nc.dram_tensor("projb", [1, C], F32, kind="ExternalInput")
    ga_d = nc.dram_tensor("gattn", [1, C], F32, kind="ExternalInput")
    ba_d = nc.dram_tensor("battn", [1, C], F32, kind="ExternalInput")
    onesq_d = nc.dram_tensor("onesq", [128, 8 * 128], F32, kind="ExternalInput")
    onesk_d = nc.dram_tensor("onesk", [128, 128], F32, kind="ExternalInput")
    invhs_d = nc.dram_tensor("invhs", [1, C], F32, kind="ExternalInput")

    with tile.TileContext(nc) as tc:
        with (
            tc.tile_pool(name="cn", bufs=1) as cn,
            tc.tile_pool(name="mid", bufs=1) as mid,
            tc.tile_pool(name="rot", bufs=3) as rot,
            tc.tile_pool(name="dram", bufs=1, space="DRAM") as dram,
        ):
            # ---- phase 0: constants ----
            vb_bc = cn.tile([128, C], F32)
            nc.sync.dma_start(out=vb_bc[:], in_=vb_d[:].to_broadcast([128, C]))
            outb_bc = cn.tile([128, C], F32)
            nc.sync.dma_start(out=outb_bc[:], in_=outb_d[:].to_broadcast([128, C]))
            pjb_bc = cn.tile([128, C], F32)
            nc.sync.dma_start(out=pjb_bc[:], in_=pjb_d[:].to_broadcast([128, C]))
            ga_bc = cn.tile([128, C], F32)
            nc.sync.dma_start(out=ga_bc[:], in_=ga_d[:].to_broadcast([128, C]))
            ba_bc = cn.tile([128, C], F32)
            nc.sync.dma_start(out=ba_bc[:], in_=ba_d[:].to_broadcast([128, C]))
            qkb_sb = cn.tile([128, 16], F32)
            nc.sync.dma_start(out=qkb_sb[:], in_=qkb_d[:])
            fcb_sb = cn.tile([128, 32], F32)
            nc.sync.dma_start(out=fcb_sb[:], in_=fcb_d[:])
            onesq_sb = cn.tile([128, 8, 128], F32)
            nc.sync.dma_start(
                out=onesq_sb[:], in_=onesq_d[:].rearrange("p (m j) -> p m j", j=128)
            )
            onesk_sb = cn.tile([128, 128], F32)
            nc.sync.dma_start(out=onesk_sb[:], in_=onesk_d[:])
            invhs_sb = cn.tile([1, C], F32)
            nc.sync.dma_start(out=invhs_sb[:], in_=invhs_d[:])
            ident = cn.tile([128, 128], BF16)
            make_identity(nc, ident[:])
            eps_sb = cn.tile([128, 1], F32)
            nc.vector.memset(eps_sb[:], LN_EPS)

            # ---- AG buffers ----
            bounce_in = dram.tile([2 * L // 2, T], BF16)       # [2048, 512]
            ag_out = dram.tile([4 * 2048, T], BF16)            # [8192, 512]

            # ---- persistent activations ----
            x_sb = [mid.tile([128, C], F32, name=f"x{t}") for t in range(TT)]
            qTn = [mid.tile([128, T], BF16, name=f"qTn{p}") for p in range(HP)]
            OT = [mid.tile([128, T], BF16, name=f"OT{p}") for p in range(HP)]
            u_sb = [mid.tile([128, C], F32, name=f"u{t}") for t in range(TT)]
            h1T = [mid.tile([128, T], BF16, name=f"h1T{c}") for c in range(CT)]
            wo_sb = mid.tile([128, CT, C], BF16)

            def layer_norm_stats(src):
                """Return (mean, rstd) APs for a [128, C] fp32 tile."""
                stats = rot.tile([128, 2, nc.vector.BN_STATS_DIM], F32, tag="lnst")
                nc.vector.bn_stats(out=stats[:, 0, :], in_=src[:, 0:512])
                nc.vector.bn_stats(out=stats[:, 1, :], in_=src[:, 512:1024])
                mv = rot.tile([128, nc.vector.BN_AGGR_DIM], F32, tag="lnmv")
                nc.vector.bn_aggr(out=mv[:], in_=stats[:])
                rstd = rot.tile([128, 1], F32, tag="lnrs")
                nc.scalar.activation(out=rstd[:], in_=mv[:, 1:2], func=AF.Sqrt,
                                     bias=eps_sb[:])
                nc.vector.reciprocal(out=rstd[:], in_=rstd[:])
                return mv[:, 0:1], rstd

            with (
                tc.tile_pool(name="pa", bufs=1) as pa,
                tc.tile_pool(name="rot1", bufs=2) as rot1,
                tc.tile_pool(name="ps1", bufs=1, space="PSUM") as ps1,
            ):
                # ---- phase 1: LN1 + transpose ----
                wv_sb = pa.tile([128, CT, C], BF16)
                nc.sync.dma_start(
                    out=wv_sb[:], in_=wv_d[:].rearrange("(k p) m -> p k m", p=128)
                )
                # out-proj weights prefetch (used in phase 5)
                nc.sync.dma_start(
                    out=wo_sb[:], in_=wo_d[:].rearrange("(k p) m -> p k m", p=128)
                )

                xlnT = [pa.tile([128, T], BF16, name=f"xlnT{c}") for c in range(CT)]
                for t in range(TT):
                    nc.sync.dma_start(out=x_sb[t][:], in_=x_d[128 * t:128 * (t + 1), :])
                    mean, rstd = layer_norm_stats(x_sb[t])
                    xb = rot1.tile([128, C], BF16, tag="xln", name=f"xln{t}")
                    nc.vector.tensor_scalar(out=xb[:], in0=x_sb[t][:], scalar1=mean,
                                            scalar2=rstd[:], op0=ALU.subtract,
                                            op1=ALU.mult)
                    for ct in range(CT):
                        ptr = ps1.tile([128, 128], BF16, tag="tr", bufs=2)
                        nc.tensor.transpose(ptr[:], xb[:, 128 * ct:128 * (ct + 1)],
                                            ident[:])
                        nc.vector.tensor_copy(
                            out=xlnT[ct][:, 128 * t:128 * (t + 1)], in_=ptr[:]
                        )

                # ---- phase 2: qk projection + l2-norm folding (k rows first) ----
                for mt in list(range(8, 16)) + list(range(8)):
                    wqk_t = rot1.tile([128, CT, 128], BF16, tag="wqk", bufs=3)
                    nc.sync.dma_start(
                        out=wqk_t[:],
                        in_=wqk_d[:, 128 * mt:128 * (mt + 1)].rearrange(
                            "(k p) m -> p k m", p=128
                        ),
                    )
                    pqk = ps1.tile([128, T], F32, tag="mm", bufs=3)
                    for kc in range(CT):
                        nc.tensor.matmul(
                            pqk[:],
                            wqk_t[:, kc, :],
                            xlnT[kc][:],
                            start=(kc == 0),
                            stop=(kc == CT - 1),
                        )
                    qk_f = rot1.tile([128, T], F32, tag="qkf")
                    nc.vector.tensor_scalar(out=qk_f[:], in0=pqk[:],
                                            scalar1=qkb_sb[:, mt:mt + 1],
                                            scalar2=None, op0=ALU.add)
                    sq = rot1.tile([128, T], F32, tag="sq")
                    nc.vector.tensor_mul(out=sq[:], in0=qk_f[:], in1=qk_f[:])
                    pn = ps1.tile([128, T], F32, tag="nrm", bufs=2)
                    ones = onesq_sb[:, mt, :] if mt < 8 else onesk_sb[:]
                    nc.tensor.matmul(pn[:], ones, sq[:], start=True, stop=True)
                    sq2 = rot1.tile([128, T], F32, tag="sqrtn")
                    nc.scalar.activation(out=sq2[:], in_=pn[:], func=AF.Sqrt)
                    rrec = rot1.tile([128, T], F32, tag="rrec")
                    nc.vector.reciprocal(out=rrec[:], in_=sq2[:])
                    if mt < 8:
                        nc.vector.tensor_mul(out=qTn[mt][:], in0=qk_f[:], in1=rrec[:])
                    else:
                        ktn = rot1.tile([128, T], BF16, tag="ktn")
                        nc.vector.tensor_mul(out=ktn[:], in0=qk_f[:], in1=rrec[:])
                        i = mt - 8
                        nc.sync.dma_start(
                            out=bounce_in[128 * i:128 * (i + 1), :], in_=ktn[:]
                        )

                # ---- phase 3: v projection ----
                for t in range(TT):
                    v_sb = rot1.tile([128, C], BF16, tag="vsb")
                    for n2 in range(2):
                        pv = ps1.tile([128, 512], F32, tag="mm", bufs=3)
                        for kc in range(CT):
                            nc.tensor.matmul(
                                pv[:],
                                xlnT[kc][:, 128 * t:128 * (t + 1)],
                                wv_sb[:, kc, 512 * n2:512 * (n2 + 1)],
                                start=(kc == 0),
                                stop=(kc == CT - 1),
                            )
                        nc.vector.tensor_tensor(
                            v_sb[:, 512 * n2:512 * (n2 + 1)], pv[:],
                            vb_bc[:, 512 * n2:512 * (n2 + 1)], ALU.add,
                        )
                    nc.sync.dma_start(
                        out=bounce_in[
                            1024 + 256 * t:1024 + 256 * (t + 1), :
                        ].rearrange("(p a) f -> p (a f)", p=128),
                        in_=v_sb[:],
                    )

                # ---- AllGather (kT_n ++ v) within each batch group ----
                nc.gpsimd.collective_compute(
                    "AllGather",
                    ALU.bypass,
                    replica_groups=[[0, 1, 2, 3], [4, 5, 6, 7]],
                    ins=[bounce_in.opt()],
                    outs=[ag_out.opt()],
                )

            # ---- phase 4: attention ----
            with (
                tc.tile_pool(name="pb", bufs=1) as pb,
                tc.tile_pool(name="rot2", bufs=2) as rot2,
                tc.tile_pool(name="ps2", bufs=1, space="PSUM") as ps2,
            ):
                # v tiles with interleaved ones column: [128, sub, head, 65]
                v_ag = []
                for r in range(RANKS):
                    vt = pb.tile([128, 4, H, HD + 1], BF16, name=f"vag{r}")
                    nc.vector.memset(vt[:], 1.0)
                    base = 2048 * r + 1024
                    for sub in range(4):
                        src = ag_out[
                            base + 256 * sub:base + 256 * (sub + 1), :
                        ].rearrange("(p a) f -> p (a f)", p=128).rearrange(
                            "p (h d) -> p h d", d=HD
                        )
                        nc.sync.dma_start(out=vt[:, sub, :, 0:HD], in_=src)
                    v_ag.append(vt)

                for hp in range(HP):
                    kpair = pb.tile([128, RANKS, T], BF16, tag="kpair", bufs=3)
                    for r in range(RANKS):
                        nc.sync.dma_start(
                            out=kpair[:, r, :],
                            in_=ag_out[
                                2048 * r + 128 * hp:2048 * r + 128 * (hp + 1), :
                            ],
                        )
                    po = [
                        ps2.tile([HD + 1, T], F32, tag=f"o{h2}", bufs=1,
                                 name=f"po{h2}")
                        for h2 in range(2)
                    ]
                    for km in range(KM):
                        r, sub = km // 4, km % 4
                        psS = ps2.tile([128, 2, T], F32, tag="s", bufs=2)
                        for h2 in range(2):
                            nc.tensor.matmul(
                                psS[:, h2, :],
                                kpair[64 * h2:64 * (h2 + 1), r,
                                      128 * sub:128 * (sub + 1)],
                                qTn[hp][64 * h2:64 * (h2 + 1), :],
                                start=True,
                                stop=True,
                            )
                        pT = rot2.tile([128, 2, T], BF16, tag="pT", bufs=4)
                        nc.scalar.activation(out=pT[:], in_=psS[:], func=AF.Exp)
                        for h2 in range(2):
                            h = 2 * hp + h2
                            nc.tensor.matmul(
                                po[h2][:],
                                v_ag[r][:, sub, h, :],
                                pT[:, h2, :],
                                start=(km == 0),
                                stop=(km == KM - 1),
                            )
                    for h2 in range(2):
                        h = 2 * hp + h2
                        rs = rot2.tile([1, T], F32, tag="rs")
                        nc.vector.tensor_copy(out=rs[:], in_=po[h2][HD:HD + 1, :])
                        pr = ps2.tile([HD, T], F32, tag="r", bufs=1)
                        nc.tensor.matmul(
                            pr[:], invhs_sb[0:1, HD * h:HD * (h + 1)], rs[:],
                            start=True, stop=True,
                        )
                        rrec = rot2.tile([HD, T], F32, tag="orec")
                        nc.vector.reciprocal(out=rrec[:], in_=pr[:])
                        nc.vector.tensor_mul(
                            out=OT[hp][64 * h2:64 * (h2 + 1), :],
                            in0=po[h2][0:HD, :],
                            in1=rrec[:],
                        )

            # ---- phase 5: out projection;  phase 6: residual + ln_attn + ln2 ----
            with (
                tc.tile_pool(name="rot3", bufs=2) as rot3,
                tc.tile_pool(name="ps3", bufs=1, space="PSUM") as ps3,
            ):
                y_sb = [rot3.tile([128, C], F32, tag="ysb", bufs=2, name=f"ysb{t}")
                        for t in range(TT)]
                for t in range(TT):
                    for n2 in range(2):
                        py = ps3.tile([128, 512], F32, tag="y", bufs=3)
                        for hp in range(HP):
                            nc.tensor.matmul(
                                py[:],
                                OT[hp][:, 128 * t:128 * (t + 1)],
                                wo_sb[:, hp, 512 * n2:512 * (n2 + 1)],
                                start=(hp == 0),
                                stop=(hp == HP - 1),
                            )
                        nc.vector.tensor_tensor(
                            y_sb[t][:, 512 * n2:512 * (n2 + 1)], py[:],
                            outb_bc[:, 512 * n2:512 * (n2 + 1)], ALU.add,
                        )
                    mean, rstd = layer_norm_stats(y_sb[t])
                    nc.vector.tensor_scalar(out=y_sb[t][:], in0=y_sb[t][:],
                                            scalar1=mean, scalar2=rstd[:],
                                            op0=ALU.subtract, op1=ALU.mult)
                    nc.vector.tensor_mul(out=y_sb[t][:], in0=y_sb[t][:], in1=ga_bc[:])
                    nc.vector.tensor_add(out=y_sb[t][:], in0=y_sb[t][:], in1=ba_bc[:])
                    nc.vector.tensor_add(out=u_sb[t][:], in0=y_sb[t][:], in1=x_sb[t][:])
                    # LN2 -> h1 (bf16) -> transposed h1T
                    mean2, rstd2 = layer_norm_stats(u_sb[t])
                    h1 = rot3.tile([128, C], BF16, tag="h1")
                    nc.vector.tensor_scalar(out=h1[:], in0=u_sb[t][:], scalar1=mean2,
                                            scalar2=rstd2[:], op0=ALU.subtract,
                                            op1=ALU.mult)
                    for ct in range(CT):
                        ptr = ps3.tile([128, 128], BF16, tag="tr", bufs=3)
                        nc.tensor.transpose(ptr[:], h1[:, 128 * ct:128 * (ct + 1)],
                                            ident[:])
                        nc.vector.tensor_copy(
                            out=h1T[ct][:, 128 * t:128 * (t + 1)], in_=ptr[:]
                        )

            # ---- phase 7: fc + gelu;  phase 8: proj + final residual ----
            with (
                tc.tile_pool(name="pd", bufs=1) as pd,
                tc.tile_pool(name="rot4", bufs=2) as rot4,
                tc.tile_pool(name="ps4", bufs=1, space="PSUM") as ps4,
            ):
                gT = [pd.tile([128, T], BF16, name=f"gT{m}") for m in range(MT_FC)]
                for mt in range(MT_FC):
                    wfc_t = rot4.tile([128, CT, 128], BF16, tag="wfc", bufs=3)
                    nc.sync.dma_start(
                        out=wfc_t[:],
                        in_=wfc_d[:, 128 * mt:128 * (mt + 1)].rearrange(
                            "(k p) m -> p k m", p=128
                        ),
                    )
                    pfc = ps4.tile([128, T], F32, tag="fc", bufs=3)
                    for kc in range(CT):
                        nc.tensor.matmul(
                            pfc[:],
                            wfc_t[:, kc, :],
                            h1T[kc][:],
                            start=(kc == 0),
                            stop=(kc == CT - 1),
                        )
                    nc.scalar.activation(out=gT[mt][:], in_=pfc[:], func=AF.Gelu,
                                         bias=fcb_sb[:, mt:mt + 1])

                for n2 in range(2):
                    ppj = [
                        ps4.tile([128, 512], F32, tag=f"pj{t}", bufs=1,
                                 name=f"ppj{t}")
                        for t in range(TT)
                    ]
                    for mt in range(MT_FC):
                        wpj_t = rot4.tile([128, 512], BF16, tag="wpj", bufs=3)
                        nc.sync.dma_start(
                            out=wpj_t[:],
                            in_=wpj_d[128 * mt:128 * (mt + 1),
                                      512 * n2:512 * (n2 + 1)],
                        )
                        for t in range(TT):
                            nc.tensor.matmul(
                                ppj[t][:],
                                gT[mt][:, 128 * t:128 * (t + 1)],
                                wpj_t[:],
                                start=(mt == 0),
                                stop=(mt == MT_FC - 1),
                            )
                    for t in range(TT):
                        o1 = rot4.tile([128, 512], F32, tag="ofin")
                        nc.vector.tensor_tensor(
                            o1[:], ppj[t][:], pjb_bc[:, 512 * n2:512 * (n2 + 1)],
                            ALU.add,
                        )
                        nc.vector.tensor_add(
                            out=o1[:], in0=o1[:],
                            in1=u_sb[t][:, 512 * n2:512 * (n2 + 1)],
                        )
                        nc.sync.dma_start(
                            out=out_d[128 * t:128 * (t + 1),
                                      512 * n2:512 * (n2 + 1)],
                            in_=o1[:],
                        )

    nc.compile()
    return nc


def _host_prep(inp):
    f32 = np.float32
    ln1_g = np.asarray(inp["ln1_g"], f32)
    ln1_b = np.asarray(inp["ln1_b"], f32)
    ln2_g = np.asarray(inp["ln2_g"], f32)
    ln2_b = np.asarray(inp["ln2_b"], f32)
    in_w = np.asarray(inp["in_proj_w"], f32)
    in_b = np.asarray(inp["in_proj_b"], f32)
    fc_w = np.asarray(inp["fc_w"], f32)
    proj_w = np.asarray(inp["proj_w"], f32)
    ls1 = np.asarray(inp["ls1"], f32)
    ls2 = np.asarray(inp["ls2"], f32)

    w_qk = in_w[:2 * C]
    w_v = in_w[2 * C:]
    p = {}
    p["wqkT"] = np.ascontiguousarray((w_qk * ln1_g[None, :]).T).astype(BF_NP)
    p["wvT"] = np.ascontiguousarray((w_v * ln1_g[None, :]).T).astype(BF_NP)
    qkb = in_b[:2 * C] + ln1_b @ w_qk.T
    p["qkb"] = np.ascontiguousarray(qkb.reshape(16, 128).T).astype(f32)
    p["vb"] = (in_b[2 * C:] + ln1_b @ w_v.T).reshape(1, C).astype(f32)
    p["woT"] = np.ascontiguousarray(np.asarray(inp["out_w"], f32).T).astype(BF_NP)
    p["outb"] = np.asarray(inp["out_b"], f32).reshape(1, C)
    p["wfcT"] = np.ascontiguousarray((fc_w * ln2_g[None, :]).T).astype(BF_NP)
    fcb = np.asarray(inp["fc_b"], f32) + ln2_b @ fc_w.T
    p["fcb"] = np.ascontiguousarray(fcb.reshape(32, 128).T).astype(f32)
    p["wprojT"] = np.ascontiguousarray((proj_w * ls2[:, None]).T).astype(BF_NP)
    p["projb"] = (ls2 * np.asarray(inp["proj_b"], f32)).reshape(1, C)
    p["gattn"] = (ls1 * np.asarray(inp["ln_attn_g"], f32)).reshape(1, C)
    p["battn"] = (ls1 * np.asarray(inp["ln_attn_b"], f32)).reshape(1, C)

    lsc = np.exp(np.minimum(np.asarray(inp["logit_scale"], f32).reshape(H),
                            np.log(100.0)))
    onesq = np.zeros((128, 8, 128), f32)
    for mt in range(8):
        for blk in range(2):
            h = 2 * mt + blk
            onesq[64 * blk:64 * (blk + 1), mt,
                  64 * blk:64 * (blk + 1)] = 1.0 / lsc[h] ** 2
    p["onesq"] = np.ascontiguousarray(onesq.reshape(128, 1024))
    onesk = np.zeros((128, 128), f32)
    onesk[:64, :64] = 1.0
    onesk[64:, 64:] = 1.0
    p["onesk"] = onesk
    hs = np.asarray(inp["head_scale"], f32)
    invhs = np.zeros((1, C), f32)
    for h in range(H):
        invhs[0, HD * h:HD * (h + 1)] = 1.0 / hs[h]
    p["invhs"] = invhs
    return p


def kernel(**inputs) -> np.ndarray:
    global _NC_CACHE, LAST_EXEC_NS, LAST_RESULTS
    if _NC_CACHE is None:
        _NC_CACHE = _build()
    nc = _NC_CACHE

    p = _host_prep(inputs)
    x = np.asarray(inputs["x"], np.float32)

    in_maps = []
    for c in range(N_CORES):
        b, r = c // RANKS, c % RANKS
        m = dict(p)
        m["x"] = np.ascontiguousarray(x[b, T * r:T * (r + 1), :])
        in_maps.append(m)

    kwargs = {}
    if TRACE:
        import os
        os.makedirs(TRACE_DIR, exist_ok=True)
        kwargs = dict(trace=True, tmpdir=TRACE_DIR)
    res = run_bass_kernel_spmd(nc, in_maps, list(range(N_CORES)), **kwargs)
    LAST_EXEC_NS = res.exec_time_ns
    LAST_RESULTS = res
    out = np.zeros((B, L, C), np.float32)
    for c in range(N_CORES):
        b, r = c // RANKS, c % RANKS
        out[b, T * r:T * (r + 1), :] = res.results[c]["out"]
    return out


# revision 5
# speedup vs baseline: 1.1351x; 1.1351x over previous
"""Trainium2 Bass kernel for nn_CustomResidualAttentionBlock (open_clip-style block).

Sharding: sequence-parallel over 8 cores. Core c owns 512 tokens
(b = c // 4, tokens [512*(c%4) : 512*(c%4+1)]). Each core computes q/k/v for its
own tokens, l2-normalizes k (and q) locally via a ones-block matmul trick, then
one AllGather per 4-core batch group distributes (kT, v) for the full 2048-key
sequence. Attention, out-proj, residuals and the MLP are fully local.

Host-side folds (exact math, done in fp32 on host):
  - ln1_g into wqkT/wvT columns; ln1_b @ W^T into the qkv biases
  - ln2_g into wfcT; ln2_b @ fc_w^T into fc bias
  - ls1 into ln_attn affine (g' = ls1*g, b' = ls1*b)
  - ls2 into proj weights/bias
  - logit_scale (clamped+exp'd) into the q-norm ones-block (1/lsc^2 entries)
  - head_scale into the rowsum-replication lhsT (1/hs entries)

All big matmuls run in bf16 with fp32 PSUM accumulation; layernorm statistics,
softmax row sums and normalization factors stay in fp32.
"""
import numpy as np
import ml_dtypes

import concourse.bass as bass
import concourse.mybir as mybir
import concourse.tile as tile
from concourse import bacc
from concourse.bass_utils import run_bass_kernel_spmd
from concourse.masks import make_identity

F32 = mybir.dt.float32
BF16 = mybir.dt.bfloat16
BF_NP = ml_dtypes.bfloat16
AF = mybir.ActivationFunctionType
ALU = mybir.AluOpType

B, L, C, H = 2, 2048, 1024, 16
HD = C // H          # 64
MLP = 4 * C          # 4096
N_CORES = 8
RANKS = 4            # cores per batch group
T = (B * L) // N_CORES  # 512 own tokens per core
TT = T // 128        # 4 token tiles
CT = C // 128        # 8 channel tiles
HP = H // 2          # 8 head pairs
KM = L // 128        # 16 key chunks
MT_FC = MLP // 128   # 32
LN_EPS = 1e-5

# exec time of the last launch (ns), populated when TRACE is set
TRACE = False
TRACE_DIR = "/tmp/bass_trace"
LAST_EXEC_NS = None
LAST_RESULTS = None

_NC_CACHE = None


def _build():
    nc = bacc.Bacc(None, target_bir_lowering=False, debug=False, num_devices=N_CORES)

    # ---- I/O ----
    x_d = nc.dram_tensor("x", [T, C], F32, kind="ExternalInput")
    out_d = nc.dram_tensor("out", [T, C], F32, kind="ExternalOutput")
    wqk_d = nc.dram_tensor("wqkT", [C, 2 * C], BF16, kind="ExternalInput")
    wv_d = nc.dram_tensor("wvT", [C, C], BF16, kind="ExternalInput")
    wo_d = nc.dram_tensor("woT", [C, C], BF16, kind="ExternalInput")
    wfc_d = nc.dram_tensor("wfcT", [C, MLP], BF16, kind="ExternalInput")
    wpj_d = nc.dram_tensor("wprojT", [MLP, C], BF16, kind="ExternalInput")
    qkb_d = nc.dram_tensor("qkb", [128, 16], F32, kind="ExternalInput")
    vb_d = nc.dram_tensor("vb", [1, C], F32, kind="ExternalInput")
    outb_d = nc.dram_tensor("outb", [1, C], F32, kind="ExternalInput")
    fcb_d = nc.dram_tensor("fcb", [128, 32], F32, kind="ExternalInput")
    pjb_d = nc.dram_tensor("projb", [1, C], F32, kind="ExternalInput")
    ga_d = nc.dram_tensor("gattn", [1, C], F32, kind="ExternalInput")
    ba_d = nc.dram_tensor("battn", [1, C], F32, kind="ExternalInput")
    onesq_d = nc.dram_tensor("onesq", [128, 8 * 128], F32, kind="ExternalInput")
    onesk_d = nc.dram_tensor("onesk", [128, 128], F32, kind="ExternalInput")
    invhs_d = nc.dram_tensor("invhs", [1, C], F32, kind="ExternalInput")

    with tile.TileContext(nc) as tc:
        with (
            tc.tile_pool(name="cn", bufs=1) as cn,
            tc.tile_pool(name="mid", bufs=1) as mid,
            tc.tile_pool(name="rot", bufs=2) as rot,
            tc.tile_pool(name="dram", bufs=1, space="DRAM") as dram,
        ):
            # ---- persistent activations ----
            x_sb = [mid.tile([128, C], F32, name=f"x{t}") for t in range(TT)]
            qTn = [mid.tile([128, T], BF16, name=f"qTn{p}") for p in range(HP)]
            OT = [mid.tile([128, T], BF16, name=f"OT{p}") for p in range(HP)]
            u_sb = [mid.tile([128, C], F32, name=f"u{t}") for t in range(TT)]
            h1T = [mid.tile([128, T], BF16, name=f"h1T{c}") for c in range(CT)]
            wo_sb = mid.tile([128, CT, C], BF16)

            # x first: LN1 is the critical path at startup
            for t in range(TT):
                nc.sync.dma_start(out=x_sb[t][:], in_=x_d[128 * t:128 * (t + 1), :])

            # ---- small constants ----
            eps_sb = cn.tile([128, 1], F32)
            nc.vector.memset(eps_sb[:], LN_EPS)
            ident = cn.tile([128, 128], BF16)
            make_identity(nc, ident[:])
            qkb_sb = cn.tile([128, 16], F32)
            nc.sync.dma_start(out=qkb_sb[:], in_=qkb_d[:])
            onesq_sb = cn.tile([128, 8, 128], F32)
            nc.sync.dma_start(
                out=onesq_sb[:], in_=onesq_d[:].rearrange("p (m j) -> p m j", j=128)
            )
            onesk_sb = cn.tile([128, 128], F32)
            nc.sync.dma_start(out=onesk_sb[:], in_=onesk_d[:])
            invhs_sb = cn.tile([1, C], F32)
            nc.sync.dma_start(out=invhs_sb[:], in_=invhs_d[:])
            fcb_sb = cn.tile([128, 32], F32)
            nc.sync.dma_start(out=fcb_sb[:], in_=fcb_d[:])

            # ---- broadcast constants ----
            vb_bc = cn.tile([128, C], F32)
            nc.sync.dma_start(out=vb_bc[:], in_=vb_d[:].to_broadcast([128, C]))
            outb_bc = cn.tile([128, C], F32)
            nc.sync.dma_start(out=outb_bc[:], in_=outb_d[:].to_broadcast([128, C]))
            pjb_bc = cn.tile([128, C], F32)
            nc.sync.dma_start(out=pjb_bc[:], in_=pjb_d[:].to_broadcast([128, C]))
            ga_bc = cn.tile([128, C], F32)
            nc.sync.dma_start(out=ga_bc[:], in_=ga_d[:].to_broadcast([128, C]))
            ba_bc = cn.tile([128, C], F32)
            nc.sync.dma_start(out=ba_bc[:], in_=ba_d[:].to_broadcast([128, C]))

            # ---- AG buffers (k and v gathered separately) ----
            bounce_k = dram.tile([1024, T], BF16)
            ag_k = dram.tile([4096, T], BF16)
            bounce_v = dram.tile([1024, T], BF16)
            ag_v = dram.tile([4096, T], BF16)

            def layer_norm_stats(src, pool):
                """Return (mean, rstd) APs for a [128, C] fp32 tile."""
                stats = pool.tile([128, 2, nc.vector.BN_STATS_DIM], F32, tag="lnst")
                nc.vector.bn_stats(out=stats[:, 0, :], in_=src[:, 0:512])
                nc.vector.bn_stats(out=stats[:, 1, :], in_=src[:, 512:1024])
                mv = pool.tile([128, nc.vector.BN_AGGR_DIM], F32, tag="lnmv")
                nc.vector.bn_aggr(out=mv[:], in_=stats[:])
                rstd = pool.tile([128, 1], F32, tag="lnrs")
                nc.scalar.activation(out=rstd[:], in_=mv[:, 1:2], func=AF.Sqrt,
                                     bias=eps_sb[:])
                nc.vector.reciprocal(out=rstd[:], in_=rstd[:])
                return mv[:, 0:1], rstd

            with (
                tc.tile_pool(name="pa", bufs=1) as pa,
                tc.tile_pool(name="rot1", bufs=2) as rot1,
                tc.tile_pool(name="ps1", bufs=1, space="PSUM") as ps1,
            ):
                # ---- phase 1: LN1 + transpose ----
                xlnT = [pa.tile([128, T], BF16, name=f"xlnT{c}") for c in range(CT)]
                with nc.named_scope("ln1_tr"):
                    for t in range(TT):
                        mean, rstd = layer_norm_stats(x_sb[t], rot1)
                        xb = rot1.tile([128, C], BF16, tag="xln", name=f"xln{t}")
                        nc.vector.tensor_scalar(out=xb[:], in0=x_sb[t][:],
                                                scalar1=mean, scalar2=rstd[:],
                                                op0=ALU.subtract, op1=ALU.mult)
                        for ct in range(CT):
                            ptr = ps1.tile([128, 128], BF16, tag="tr", bufs=2)
                            nc.tensor.transpose(
                                ptr[:], xb[:, 128 * ct:128 * (ct + 1)], ident[:]
                            )
                            nc.vector.tensor_copy(
                                out=xlnT[ct][:, 128 * t:128 * (t + 1)], in_=ptr[:]
                            )

                def qk_tile(mt):
                    """Project + l2-normalize one 128-row tile of q or k."""
                    wqk_t = rot1.tile([128, CT, 128], BF16, tag="wqk", bufs=3,
                                      name=f"wqk{mt}")
                    nc.sync.dma_start(
                        out=wqk_t[:],
                        in_=wqk_d[:, 128 * mt:128 * (mt + 1)].rearrange(
                            "(k p) m -> p k m", p=128
                        ),
                    )
                    pqk = ps1.tile([128, T], F32, tag="mm", bufs=3, name=f"pqk{mt}")
                    for kc in range(CT):
                        nc.tensor.matmul(
                            pqk[:], wqk_t[:, kc, :], xlnT[kc][:],
                            start=(kc == 0), stop=(kc == CT - 1),
                        )
                    qk_f = rot1.tile([128, T], F32, tag="qkf", name=f"qkf{mt}")
                    nc.vector.tensor_scalar(out=qk_f[:], in0=pqk[:],
                                            scalar1=qkb_sb[:, mt:mt + 1],
                                            scalar2=None, op0=ALU.add)
                    sq = rot1.tile([128, T], F32, tag="sq", name=f"sq{mt}")
                    nc.vector.tensor_mul(out=sq[:], in0=qk_f[:], in1=qk_f[:])
                    pn = ps1.tile([128, T], F32, tag="nrm", bufs=2, name=f"pn{mt}")
                    ones = onesq_sb[:, mt, :] if mt < 8 else onesk_sb[:]
                    nc.tensor.matmul(pn[:], ones, sq[:], start=True, stop=True)
                    sq2 = rot1.tile([128, T], F32, tag="sqrtn", name=f"sqn{mt}")
                    nc.scalar.activation(out=sq2[:], in_=pn[:], func=AF.Sqrt)
                    rrec = rot1.tile([128, T], F32, tag="rrec", name=f"rrec{mt}")
                    nc.vector.reciprocal(out=rrec[:], in_=sq2[:])
                    return qk_f, rrec

                # ---- phase 2k: k rows -> bounce -> AG-k ----
                with nc.named_scope("kproj"):
                    for mt in range(8, 16):
                        qk_f, rrec = qk_tile(mt)
                        i = mt - 8
                        ktn = rot1.tile([128, T], BF16, tag="ktn", name=f"ktn{i}")
                        nc.vector.tensor_mul(out=ktn[:], in0=qk_f[:], in1=rrec[:])
                        nc.sync.dma_start(
                            out=bounce_k[128 * i:128 * (i + 1), :], in_=ktn[:]
                        )
                nc.gpsimd.collective_compute(
                    "AllGather", ALU.bypass,
                    replica_groups=[[0, 1, 2, 3], [4, 5, 6, 7]],
                    ins=[bounce_k.opt()], outs=[ag_k.opt()],
                )

                # ---- phase 3: v -> bounce -> AG-v ----
                wv_sb = pa.tile([128, CT, C], BF16)
                nc.sync.dma_start(
                    out=wv_sb[:], in_=wv_d[:].rearrange("(k p) m -> p k m", p=128)
                )
                with nc.named_scope("vproj"):
                    for t in range(TT):
                        v_sb = rot1.tile([128, C], BF16, tag="vsb", name=f"vsb{t}")
                        for n2 in range(2):
                            pv = ps1.tile([128, 512], F32, tag="mm", bufs=3,
                                          name=f"pv{t}{n2}")
                            for kc in range(CT):
                                nc.tensor.matmul(
                                    pv[:],
                                    xlnT[kc][:, 128 * t:128 * (t + 1)],
                                    wv_sb[:, kc, 512 * n2:512 * (n2 + 1)],
                                    start=(kc == 0), stop=(kc == CT - 1),
                                )
                            nc.vector.tensor_tensor(
                                v_sb[:, 512 * n2:512 * (n2 + 1)], pv[:],
                                vb_bc[:, 512 * n2:512 * (n2 + 1)], ALU.add,
                            )
                        nc.sync.dma_start(
                            out=bounce_v[256 * t:256 * (t + 1), :].rearrange(
                                "(p a) f -> p (a f)", p=128
                            ),
                            in_=v_sb[:],
                        )
                nc.gpsimd.collective_compute(
                    "AllGather", ALU.bypass,
                    replica_groups=[[0, 1, 2, 3], [4, 5, 6, 7]],
                    ins=[bounce_v.opt()], outs=[ag_v.opt()],
                )

                # out-proj weights prefetch (used in phase 5)
                nc.sync.dma_start(
                    out=wo_sb[:], in_=wo_d[:].rearrange("(k p) m -> p k m", p=128)
                )

                # ---- phase 2q: q rows ----
                with nc.named_scope("qproj"):
                    for mt in range(8):
                        qk_f, rrec = qk_tile(mt)
                        nc.vector.tensor_mul(out=qTn[mt][:], in0=qk_f[:], in1=rrec[:])

            # ---- phase 4: attention ----
            with (
                tc.tile_pool(name="pb", bufs=1) as pb,
                tc.tile_pool(name="rot2", bufs=2) as rot2,
                tc.tile_pool(name="ps2", bufs=1, space="PSUM") as ps2,
            ):
                # v tiles with interleaved ones column: [128, sub, head, 65]
                v_ag = []
                with nc.named_scope("vload"):
                    for r in range(RANKS):
                        vt = pb.tile([128, 4, H, HD + 1], BF16, name=f"vag{r}")
                        nc.vector.memset(vt[:], 1.0)
                        for sub in range(4):
                            src = ag_v[
                                1024 * r + 256 * sub:1024 * r + 256 * (sub + 1), :
                            ].rearrange("(p a) f -> p (a f)", p=128).rearrange(
                                "p (h d) -> p h d", d=HD
                            )
                            nc.sync.dma_start(out=vt[:, sub, :, 0:HD], in_=src)
                        v_ag.append(vt)

                ag_k_r = ag_k[:].rearrange("(r m p) f -> p r m f", r=RANKS, p=128)
                with nc.named_scope("attn"):
                    for hp in range(HP):
                        kpair = pb.tile([128, RANKS, T], BF16, tag="kpair", bufs=3,
                                        name=f"kpair{hp}")
                        nc.sync.dma_start(out=kpair[:], in_=ag_k_r[:, :, hp, :])
                        po = [
                            ps2.tile([HD + 1, T], F32, tag=f"o{h2}", bufs=1,
                                     name=f"po{h2}")
                            for h2 in range(2)
                        ]
                        for km in range(KM):
                            r, sub = km // 4, km % 4
                            psS = ps2.tile([128, 2, T], F32, tag="s", bufs=2,
                                           name=f"psS{km}")
                            for h2 in range(2):
                                nc.tensor.matmul(
                                    psS[:, h2, :],
                                    kpair[64 * h2:64 * (h2 + 1), r,
                                          128 * sub:128 * (sub + 1)],
                                    qTn[hp][64 * h2:64 * (h2 + 1), :],
                                    start=True, stop=True,
                                )
                            pT = rot2.tile([128, 2, T], BF16, tag="pT", bufs=4,
                                           name=f"pT{km}")
                            nc.scalar.activation(out=pT[:], in_=psS[:], func=AF.Exp)
                            for h2 in range(2):
                                h = 2 * hp + h2
                                nc.tensor.matmul(
                                    po[h2][:],
                                    v_ag[r][:, sub, h, :],
                                    pT[:, h2, :],
                                    start=(km == 0), stop=(km == KM - 1),
                                )
                        for h2 in range(2):
                            h = 2 * hp + h2
                            rs = rot2.tile([1, T], F32, tag="rs")
                            nc.vector.tensor_copy(out=rs[:], in_=po[h2][HD:HD + 1, :])
                            pr = ps2.tile([HD, T], F32, tag="r", bufs=1)
                            nc.tensor.matmul(
                                pr[:], invhs_sb[0:1, HD * h:HD * (h + 1)], rs[:],
                                start=True, stop=True,
                            )
                            rrec = rot2.tile([HD, T], F32, tag="orec")
                            nc.vector.reciprocal(out=rrec[:], in_=pr[:])
                            nc.vector.tensor_mul(
                                out=OT[hp][64 * h2:64 * (h2 + 1), :],
                                in0=po[h2][0:HD, :],
                                in1=rrec[:],
                            )

            # ---- phase 5: out projection;  phase 6: residual + ln_attn + ln2 ----
            with (
                tc.tile_pool(name="rot3", bufs=2) as rot3,
                tc.tile_pool(name="ps3", bufs=1, space="PSUM") as ps3,
            ):
                with nc.named_scope("outproj_ln"):
                    for t in range(TT):
                        y_sb = rot3.tile([128, C], F32, tag="ysb", bufs=2,
                                         name=f"ysb{t}")
                        for n2 in range(2):
                            py = ps3.tile([128, 512], F32, tag="y", bufs=3,
                                          name=f"py{t}{n2}")
                            for hp in range(HP):
                                nc.tensor.matmul(
                                    py[:],
                                    OT[hp][:, 128 * t:128 * (t + 1)],
                                    wo_sb[:, hp, 512 * n2:512 * (n2 + 1)],
                                    start=(hp == 0), stop=(hp == HP - 1),
                                )
                            nc.vector.tensor_tensor(
                                y_sb[:, 512 * n2:512 * (n2 + 1)], py[:],
                                outb_bc[:, 512 * n2:512 * (n2 + 1)], ALU.add,
                            )
                        mean, rstd = layer_norm_stats(y_sb, rot3)
                        nc.vector.tensor_scalar(out=y_sb[:], in0=y_sb[:],
                                                scalar1=mean, scalar2=rstd[:],
                                                op0=ALU.subtract, op1=ALU.mult)
                        nc.vector.tensor_mul(out=y_sb[:], in0=y_sb[:], in1=ga_bc[:])
                        nc.vector.tensor_add(out=y_sb[:], in0=y_sb[:], in1=ba_bc[:])
                        nc.vector.tensor_add(out=u_sb[t][:], in0=y_sb[:],
                                             in1=x_sb[t][:])
                        # LN2 -> h1 (bf16) -> transposed h1T
                        mean2, rstd2 = layer_norm_stats(u_sb[t], rot3)
                        h1 = rot3.tile([128, C], BF16, tag="h1")
                        nc.vector.tensor_scalar(out=h1[:], in0=u_sb[t][:],
                                                scalar1=mean2, scalar2=rstd2[:],
                                                op0=ALU.subtract, op1=ALU.mult)
                        for ct in range(CT):
                            ptr = ps3.tile([128, 128], BF16, tag="tr", bufs=3)
                            nc.tensor.transpose(
                                ptr[:], h1[:, 128 * ct:128 * (ct + 1)], ident[:]
                            )
                            nc.vector.tensor_copy(
                                out=h1T[ct][:, 128 * t:128 * (t + 1)], in_=ptr[:]
                            )

            # ---- phase 7: fc + gelu;  phase 8: proj + final residual ----
            with (
                tc.tile_pool(name="pd", bufs=1) as pd,
                tc.tile_pool(name="rot4", bufs=2) as rot4,
                tc.tile_pool(name="ps4", bufs=1, space="PSUM") as ps4,
            ):
                gT = [pd.tile([128, T], BF16, name=f"gT{m}") for m in range(MT_FC)]
                with nc.named_scope("fc"):
                    for mt in range(MT_FC):
                        wfc_t = rot4.tile([128, CT, 128], BF16, tag="wfc", bufs=3,
                                          name=f"wfc{mt}")
                        nc.sync.dma_start(
                            out=wfc_t[:],
                            in_=wfc_d[:, 128 * mt:128 * (mt + 1)].rearrange(
                                "(k p) m -> p k m", p=128
                            ),
                        )
                        pfc = ps4.tile([128, T], F32, tag="fc", bufs=3,
                                       name=f"pfc{mt}")
                        for kc in range(CT):
                            nc.tensor.matmul(
                                pfc[:], wfc_t[:, kc, :], h1T[kc][:],
                                start=(kc == 0), stop=(kc == CT - 1),
                            )
                        nc.scalar.activation(out=gT[mt][:], in_=pfc[:], func=AF.Gelu,
                                             bias=fcb_sb[:, mt:mt + 1])

                with nc.named_scope("proj"):
                    for n2 in range(2):
                        ppj = [
                            ps4.tile([128, 512], F32, tag=f"pj{t}", bufs=1,
                                     name=f"ppj{t}")
                            for t in range(TT)
                        ]
                        for mt in range(MT_FC):
                            wpj_t = rot4.tile([128, 512], BF16, tag="wpj", bufs=3,
                                              name=f"wpj{n2}_{mt}")
                            nc.sync.dma_start(
                                out=wpj_t[:],
                                in_=wpj_d[128 * mt:128 * (mt + 1),
                                          512 * n2:512 * (n2 + 1)],
                            )
                            for t in range(TT):
                                nc.tensor.matmul(
                                    ppj[t][:],
                                    gT[mt][:, 128 * t:128 * (t + 1)],
                                    wpj_t[:],
                                    start=(mt == 0), stop=(mt == MT_FC - 1),
                                )
                        for t in range(TT):
                            o1 = rot4.tile([128, 512], F32, tag="ofin",
                                           name=f"of{n2}_{t}")
                            nc.vector.tensor_tensor(
                                o1[:], ppj[t][:],
                                pjb_bc[:, 512 * n2:512 * (n2 + 1)], ALU.add,
                            )
                            nc.vector.tensor_add(
                                out=o1[:], in0=o1[:],
                                in1=u_sb[t][:, 512 * n2:512 * (n2 + 1)],
                            )
                            nc.sync.dma_start(
                                out=out_d[128 * t:128 * (t + 1),
                                          512 * n2:512 * (n2 + 1)],
                                in_=o1[:],
                            )

    nc.compile()
    return nc


def _host_prep(inp):
    f32 = np.float32
    ln1_g = np.asarray(inp["ln1_g"], f32)
    ln1_b = np.asarray(inp["ln1_b"], f32)
    ln2_g = np.asarray(inp["ln2_g"], f32)
    ln2_b = np.asarray(inp["ln2_b"], f32)
    in_w = np.asarray(inp["in_proj_w"], f32)
    in_b = np.asarray(inp["in_proj_b"], f32)
    fc_w = np.asarray(inp["fc_w"], f32)
    proj_w = np.asarray(inp["proj_w"], f32)
    ls1 = np.asarray(inp["ls1"], f32)
    ls2 = np.asarray(inp["ls2"], f32)

    w_qk = in_w[:2 * C]
    w_v = in_w[2 * C:]
    p = {}
    p["wqkT"] = np.ascontiguousarray((w_qk * ln1_g[None, :]).T).astype(BF_NP)
    p["wvT"] = np.ascontiguousarray((w_v * ln1_g[None, :]).T).astype(BF_NP)
    qkb = in_b[:2 * C] + ln1_b @ w_qk.T
    p["qkb"] = np.ascontiguousarray(qkb.reshape(16, 128).T).astype(f32)
    p["vb"] = (in_b[2 * C:] + ln1_b @ w_v.T).reshape(1, C).astype(f32)
    p["woT"] = np.ascontiguousarray(np.asarray(inp["out_w"], f32).T).astype(BF_NP)
    p["outb"] = np.asarray(inp["out_b"], f32).reshape(1, C)
    p["wfcT"] = np.ascontiguousarray((fc_w * ln2_g[None, :]).T).astype(BF_NP)
    fcb = np.asarray(inp["fc_b"], f32) + ln2_b @ fc_w.T
    p["fcb"] = np.ascontiguousarray(fcb.reshape(32, 128).T).astype(f32)
    p["wprojT"] = np.ascontiguousarray((proj_w * ls2[:, None]).T).astype(BF_NP)
    p["projb"] = (ls2 * np.asarray(inp["proj_b"], f32)).reshape(1, C)
    p["gattn"] = (ls1 * np.asarray(inp["ln_attn_g"], f32)).reshape(1, C)
    p["battn"] = (ls1 * np.asarray(inp["ln_attn_b"], f32)).reshape(1, C)

    lsc = np.exp(np.minimum(np.asarray(inp["logit_scale"], f32).reshape(H),
                            np.log(100.0)))
    onesq = np.zeros((128, 8, 128), f32)
    for mt in range(8):
        for blk in range(2):
            h = 2 * mt + blk
            onesq[64 * blk:64 * (blk + 1), mt,
                  64 * blk:64 * (blk + 1)] = 1.0 / lsc[h] ** 2
    p["onesq"] = np.ascontiguousarray(onesq.reshape(128, 1024))
    onesk = np.zeros((128, 128), f32)
    onesk[:64, :64] = 1.0
    onesk[64:, 64:] = 1.0
    p["onesk"] = onesk
    hs = np.asarray(inp["head_scale"], f32)
    invhs = np.zeros((1, C), f32)
    for h in range(H):
        invhs[0, HD * h:HD * (h + 1)] = 1.0 / hs[h]
    p["invhs"] = invhs
    return p


def kernel(**inputs) -> np.ndarray:
    global _NC_CACHE, LAST_EXEC_NS, LAST_RESULTS
    if _NC_CACHE is None:
        _NC_CACHE = _build()
    nc = _NC_CACHE

    p = _host_prep(inputs)
    x = np.asarray(inputs["x"], np.float32)

    in_maps = []
    for c in range(N_CORES):
        b, r = c // RANKS, c % RANKS
        m = dict(p)
        m["x"] = np.ascontiguousarray(x[b, T * r:T * (r + 1), :])
        in_maps.append(m)

    kwargs = {}
    if TRACE:
        import os
        os.makedirs(TRACE_DIR, exist_ok=True)
        kwargs = dict(trace=True, tmpdir=TRACE_DIR)
    res = run_bass_kernel_spmd(nc, in_maps, list(range(N_CORES)), **kwargs)
    LAST_EXEC_NS = res.exec_time_ns
    LAST_RESULTS = res
    out = np.zeros((B, L, C), np.float32)
    for c in range(N_CORES):
        b, r = c // RANKS, c % RANKS
        out[b, T * r:T * (r + 1), :] = res.results[c]["out"]
    return out


# revision 7
# speedup vs baseline: 1.1454x; 1.0090x over previous
"""Trainium2 Bass kernel for nn_CustomResidualAttentionBlock (open_clip-style block).

Sharding: sequence-parallel over 8 cores. Core c owns 512 tokens
(b = c // 4, tokens [512*(c%4) : 512*(c%4+1)]). Each core computes q/k/v for its
own tokens, l2-normalizes k (and q) locally via a ones-block matmul trick, then
one AllGather per 4-core batch group distributes (kT, v) for the full 2048-key
sequence. Attention, out-proj, residuals and the MLP are fully local.

Host-side folds (exact math, done in fp32 on host):
  - ln1_g into wqkT/wvT columns; ln1_b @ W^T into the qkv biases
  - ln2_g into wfcT; ln2_b @ fc_w^T into fc bias
  - ls1 into ln_attn affine (g' = ls1*g, b' = ls1*b)
  - ls2 into proj weights/bias
  - logit_scale (clamped+exp'd) into the q-norm ones-block (1/lsc^2 entries)
  - head_scale into the rowsum-replication lhsT (1/hs entries)

All big matmuls run in bf16 with fp32 PSUM accumulation; layernorm statistics,
softmax row sums and normalization factors stay in fp32.
"""
import numpy as np
import ml_dtypes

import concourse.bass as bass
import concourse.mybir as mybir
import concourse.tile as tile
from concourse import bacc
from concourse.bass_utils import run_bass_kernel_spmd
from concourse.masks import make_identity

F32 = mybir.dt.float32
BF16 = mybir.dt.bfloat16
BF_NP = ml_dtypes.bfloat16
AF = mybir.ActivationFunctionType
ALU = mybir.AluOpType

B, L, C, H = 2, 2048, 1024, 16
HD = C // H          # 64
MLP = 4 * C          # 4096
N_CORES = 8
RANKS = 4            # cores per batch group
T = (B * L) // N_CORES  # 512 own tokens per core
TT = T // 128        # 4 token tiles
CT = C // 128        # 8 channel tiles
HP = H // 2          # 8 head pairs
KM = L // 128        # 16 key chunks
MT_FC = MLP // 128   # 32
LN_EPS = 1e-5

# exec time of the last launch (ns), populated when TRACE is set
TRACE = False
TRACE_DIR = "/tmp/bass_trace"
LAST_EXEC_NS = None
LAST_RESULTS = None

_NC_CACHE = None


def _build():
    nc = bacc.Bacc(None, target_bir_lowering=False, debug=False, num_devices=N_CORES)

    # ---- I/O ----
    x_d = nc.dram_tensor("x", [T, C], F32, kind="ExternalInput")
    out_d = nc.dram_tensor("out", [T, C], F32, kind="ExternalOutput")
    wqk_d = nc.dram_tensor("wqkT", [C, 2 * C], BF16, kind="ExternalInput")
    wv_d = nc.dram_tensor("wvT", [C, C], BF16, kind="ExternalInput")
    wo_d = nc.dram_tensor("woT", [C, C], BF16, kind="ExternalInput")
    wfc_d = nc.dram_tensor("wfcT", [C, MLP], BF16, kind="ExternalInput")
    wpj_d = nc.dram_tensor("wprojT", [MLP, C], BF16, kind="ExternalInput")
    qkb_d = nc.dram_tensor("qkb", [128, 16], F32, kind="ExternalInput")
    vb_d = nc.dram_tensor("vb", [1, C], F32, kind="ExternalInput")
    outb_d = nc.dram_tensor("outb", [1, C], F32, kind="ExternalInput")
    fcb_d = nc.dram_tensor("fcb", [128, 32], F32, kind="ExternalInput")
    pjb_d = nc.dram_tensor("projb", [1, C], F32, kind="ExternalInput")
    ga_d = nc.dram_tensor("gattn", [1, C], F32, kind="ExternalInput")
    ba_d = nc.dram_tensor("battn", [1, C], F32, kind="ExternalInput")
    onesq_d = nc.dram_tensor("onesq", [128, 8 * 128], F32, kind="ExternalInput")
    onesk_d = nc.dram_tensor("onesk", [128, 128], F32, kind="ExternalInput")
    invhs_d = nc.dram_tensor("invhs", [1, C], F32, kind="ExternalInput")

    with tile.TileContext(nc) as tc:
        with (
            tc.tile_pool(name="cn", bufs=1) as cn,
            tc.tile_pool(name="mid", bufs=1) as mid,
            tc.tile_pool(name="rot", bufs=2) as rot,
            tc.tile_pool(name="dram", bufs=1, space="DRAM") as dram,
        ):
            # ---- persistent activations ----
            x_sb = [mid.tile([128, C], F32, name=f"x{t}") for t in range(TT)]
            qTn = [mid.tile([128, T], BF16, name=f"qTn{p}") for p in range(HP)]
            OT = [mid.tile([128, T], BF16, name=f"OT{p}") for p in range(HP)]
            u_sb = [mid.tile([128, C], F32, name=f"u{t}") for t in range(TT)]
            h1T = [mid.tile([128, T], BF16, name=f"h1T{c}") for c in range(CT)]
            wo_sb = mid.tile([128, CT, C], BF16)

            # x first: LN1 is the critical path at startup
            for t in range(TT):
                nc.sync.dma_start(out=x_sb[t][:], in_=x_d[128 * t:128 * (t + 1), :])

            # ---- small constants ----
            eps_sb = cn.tile([128, 1], F32)
            nc.vector.memset(eps_sb[:], LN_EPS)
            ident = cn.tile([128, 128], BF16)
            make_identity(nc, ident[:])
            qkb_sb = cn.tile([128, 16], F32)
            nc.sync.dma_start(out=qkb_sb[:], in_=qkb_d[:])
            onesq_sb = cn.tile([128, 8, 128], F32)
            nc.sync.dma_start(
                out=onesq_sb[:], in_=onesq_d[:].rearrange("p (m j) -> p m j", j=128)
            )
            onesk_sb = cn.tile([128, 128], F32)
            nc.sync.dma_start(out=onesk_sb[:], in_=onesk_d[:])
            invhs_sb = cn.tile([1, C], F32)
            nc.sync.dma_start(out=invhs_sb[:], in_=invhs_d[:])
            fcb_sb = cn.tile([128, 32], F32)
            nc.sync.dma_start(out=fcb_sb[:], in_=fcb_d[:])

            # ---- broadcast constants ----
            vb_bc = cn.tile([128, C], F32)
            nc.sync.dma_start(out=vb_bc[:], in_=vb_d[:].to_broadcast([128, C]))
            outb_bc = cn.tile([128, C], F32)
            nc.sync.dma_start(out=outb_bc[:], in_=outb_d[:].to_broadcast([128, C]))
            pjb_bc = cn.tile([128, C], F32)
            nc.sync.dma_start(out=pjb_bc[:], in_=pjb_d[:].to_broadcast([128, C]))
            ga_bc = cn.tile([128, C], F32)
            nc.sync.dma_start(out=ga_bc[:], in_=ga_d[:].to_broadcast([128, C]))
            ba_bc = cn.tile([128, C], F32)
            nc.sync.dma_start(out=ba_bc[:], in_=ba_d[:].to_broadcast([128, C]))

            # ---- AG buffers (k and v gathered separately) ----
            bounce_k = dram.tile([1024, T], BF16)
            ag_k = dram.tile([4096, T], BF16)
            bounce_v = dram.tile([1024, T], BF16)
            ag_v = dram.tile([4096, T], BF16)

            def layer_norm_stats(src, pool):
                """Return (mean, rstd) APs for a [128, C] fp32 tile."""
                stats = pool.tile([128, 2, nc.vector.BN_STATS_DIM], F32, tag="lnst")
                nc.vector.bn_stats(out=stats[:, 0, :], in_=src[:, 0:512])
                nc.vector.bn_stats(out=stats[:, 1, :], in_=src[:, 512:1024])
                mv = pool.tile([128, nc.vector.BN_AGGR_DIM], F32, tag="lnmv")
                nc.vector.bn_aggr(out=mv[:], in_=stats[:])
                rstd = pool.tile([128, 1], F32, tag="lnrs")
                nc.scalar.activation(out=rstd[:], in_=mv[:, 1:2], func=AF.Sqrt,
                                     bias=eps_sb[:])
                nc.vector.reciprocal(out=rstd[:], in_=rstd[:])
                return mv[:, 0:1], rstd

            with (
                tc.tile_pool(name="pa", bufs=1) as pa,
                tc.tile_pool(name="rot1", bufs=2) as rot1,
                tc.tile_pool(name="ps1", bufs=1, space="PSUM") as ps1,
            ):
                # ---- phase 1: LN1 + transpose ----
                xlnT = [pa.tile([128, T], BF16, name=f"xlnT{c}") for c in range(CT)]
                with nc.named_scope("ln1_tr"):
                    for t in range(TT):
                        mean, rstd = layer_norm_stats(x_sb[t], rot1)
                        xb = rot1.tile([128, C], BF16, tag="xln", name=f"xln{t}")
                        nc.vector.tensor_scalar(out=xb[:], in0=x_sb[t][:],
                                                scalar1=mean, scalar2=rstd[:],
                                                op0=ALU.subtract, op1=ALU.mult)
                        for ct in range(CT):
                            ptr = ps1.tile([128, 128], BF16, tag="tr", bufs=2)
                            nc.tensor.transpose(
                                ptr[:], xb[:, 128 * ct:128 * (ct + 1)], ident[:]
                            )
                            nc.vector.tensor_copy(
                                out=xlnT[ct][:, 128 * t:128 * (t + 1)], in_=ptr[:]
                            )

                def qk_tile(mt):
                    """Project + l2-normalize one 128-row tile of q or k."""
                    wqk_t = rot1.tile([128, CT, 128], BF16, tag="wqk", bufs=3,
                                      name=f"wqk{mt}")
                    nc.sync.dma_start(
                        out=wqk_t[:],
                        in_=wqk_d[:, 128 * mt:128 * (mt + 1)].rearrange(
                            "(k p) m -> p k m", p=128
                        ),
                    )
                    pqk = ps1.tile([128, T], F32, tag="mm", bufs=3, name=f"pqk{mt}")
                    for kc in range(CT):
                        nc.tensor.matmul(
                            pqk[:], wqk_t[:, kc, :], xlnT[kc][:],
                            start=(kc == 0), stop=(kc == CT - 1),
                        )
                    qk_f = rot1.tile([128, T], F32, tag="qkf", name=f"qkf{mt}")
                    nc.vector.tensor_scalar(out=qk_f[:], in0=pqk[:],
                                            scalar1=qkb_sb[:, mt:mt + 1],
                                            scalar2=None, op0=ALU.add)
                    sq = rot1.tile([128, T], F32, tag="sq", name=f"sq{mt}")
                    nc.scalar.activation(out=sq[:], in_=pqk[:], func=AF.Square,
                                         bias=qkb_sb[:, mt:mt + 1])
                    pn = ps1.tile([128, T], F32, tag="nrm", bufs=2, name=f"pn{mt}")
                    ones = onesq_sb[:, mt, :] if mt < 8 else onesk_sb[:]
                    nc.tensor.matmul(pn[:], ones, sq[:], start=True, stop=True)
                    sq2 = rot1.tile([128, T], F32, tag="sqrtn", name=f"sqn{mt}")
                    nc.scalar.activation(out=sq2[:], in_=pn[:], func=AF.Sqrt)
                    rrec = rot1.tile([128, T], F32, tag="rrec", name=f"rrec{mt}")
                    nc.vector.reciprocal(out=rrec[:], in_=sq2[:])
                    return qk_f, rrec

                # ---- phase 2k: k rows -> bounce -> AG-k ----
                with nc.named_scope("kproj"):
                    for mt in range(8, 16):
                        qk_f, rrec = qk_tile(mt)
                        i = mt - 8
                        ktn = rot1.tile([128, T], BF16, tag="ktn", name=f"ktn{i}")
                        nc.vector.tensor_mul(out=ktn[:], in0=qk_f[:], in1=rrec[:])
                        nc.sync.dma_start(
                            out=bounce_k[128 * i:128 * (i + 1), :], in_=ktn[:]
                        )
                nc.gpsimd.collective_compute(
                    "AllGather", ALU.bypass,
                    replica_groups=[[0, 1, 2, 3], [4, 5, 6, 7]],
                    ins=[bounce_k.opt()], outs=[ag_k.opt()],
                )

                # ---- phase 3: v -> bounce -> AG-v ----
                wv_sb = pa.tile([128, CT, C], BF16)
                nc.sync.dma_start(
                    out=wv_sb[:], in_=wv_d[:].rearrange("(k p) m -> p k m", p=128)
                )
                with nc.named_scope("vproj"):
                    for t in range(TT):
                        v_sb = rot1.tile([128, C], BF16, tag="vsb", name=f"vsb{t}")
                        for n2 in range(2):
                            pv = ps1.tile([128, 512], F32, tag="mm", bufs=3,
                                          name=f"pv{t}{n2}")
                            for kc in range(CT):
                                nc.tensor.matmul(
                                    pv[:],
                                    xlnT[kc][:, 128 * t:128 * (t + 1)],
                                    wv_sb[:, kc, 512 * n2:512 * (n2 + 1)],
                                    start=(kc == 0), stop=(kc == CT - 1),
                                )
                            nc.vector.tensor_tensor(
                                v_sb[:, 512 * n2:512 * (n2 + 1)], pv[:],
                                vb_bc[:, 512 * n2:512 * (n2 + 1)], ALU.add,
                            )
                        nc.sync.dma_start(
                            out=bounce_v[256 * t:256 * (t + 1), :].rearrange(
                                "(p a) f -> p (a f)", p=128
                            ),
                            in_=v_sb[:],
                        )
                nc.gpsimd.collective_compute(
                    "AllGather", ALU.bypass,
                    replica_groups=[[0, 1, 2, 3], [4, 5, 6, 7]],
                    ins=[bounce_v.opt()], outs=[ag_v.opt()],
                )

                # out-proj weights prefetch (used in phase 5)
                nc.sync.dma_start(
                    out=wo_sb[:], in_=wo_d[:].rearrange("(k p) m -> p k m", p=128)
                )

                # ---- phase 2q: q rows ----
                with nc.named_scope("qproj"):
                    for mt in range(8):
                        qk_f, rrec = qk_tile(mt)
                        nc.vector.tensor_mul(out=qTn[mt][:], in0=qk_f[:], in1=rrec[:])

            # ---- phase 4: attention ----
            with (
                tc.tile_pool(name="pb", bufs=1) as pb,
                tc.tile_pool(name="rot2", bufs=2) as rot2,
                tc.tile_pool(name="ps2", bufs=1, space="PSUM") as ps2,
            ):
                # v tiles with interleaved ones column: [128, sub, head, 65]
                v_ag = []
                with nc.named_scope("vload"):
                    for r in range(RANKS):
                        vt = pb.tile([128, 4, H, HD + 1], BF16, name=f"vag{r}")
                        nc.vector.memset(vt[:], 1.0)
                        for sub in range(4):
                            src = ag_v[
                                1024 * r + 256 * sub:1024 * r + 256 * (sub + 1), :
                            ].rearrange("(p a) f -> p (a f)", p=128).rearrange(
                                "p (h d) -> p h d", d=HD
                            )
                            nc.sync.dma_start(out=vt[:, sub, :, 0:HD], in_=src)
                        v_ag.append(vt)

                ag_k_r = ag_k[:].rearrange("(r m p) f -> p r m f", r=RANKS, p=128)
                with nc.named_scope("attn"):
                    for hp in range(HP):
                        kpair = pb.tile([128, RANKS, T], BF16, tag="kpair", bufs=3,
                                        name=f"kpair{hp}")
                        nc.sync.dma_start(out=kpair[:], in_=ag_k_r[:, :, hp, :])
                        po = [
                            ps2.tile([HD + 1, T], F32, tag=f"o{h2}", bufs=2,
                                     name=f"po{h2}")
                            for h2 in range(2)
                        ]
                        for km in range(KM):
                            r, sub = km // 4, km % 4
                            psS = ps2.tile([128, 2, T], F32, tag="s", bufs=2,
                                           name=f"psS{km}")
                            for h2 in range(2):
                                nc.tensor.matmul(
                                    psS[:, h2, :],
                                    kpair[64 * h2:64 * (h2 + 1), r,
                                          128 * sub:128 * (sub + 1)],
                                    qTn[hp][64 * h2:64 * (h2 + 1), :],
                                    start=True, stop=True,
                                )
                            pT = rot2.tile([128, 2, T], BF16, tag="pT", bufs=4,
                                           name=f"pT{km}")
                            nc.scalar.activation(out=pT[:], in_=psS[:], func=AF.Exp)
                            for h2 in range(2):
                                h = 2 * hp + h2
                                nc.tensor.matmul(
                                    po[h2][:],
                                    v_ag[r][:, sub, h, :],
                                    pT[:, h2, :],
                                    start=(km == 0), stop=(km == KM - 1),
                                )
                        for h2 in range(2):
                            h = 2 * hp + h2
                            # recip(rowsum) * head_scale, then broadcast to 64
                            # partitions via DMA and scale O
                            rs = rot2.tile([1, T], F32, tag="rs")
                            nc.vector.tensor_copy(out=rs[:], in_=po[h2][HD:HD + 1, :])
                            pr = ps2.tile([128, 2, T], F32, tag="s", bufs=2,
                                          name=f"pr{h2}")
                            nc.tensor.matmul(
                                pr[0:HD, 0, :],
                                invhs_sb[0:1, HD * h:HD * (h + 1)], rs[:],
                                start=True, stop=True,
                            )
                            rrec = rot2.tile([HD, T], F32, tag="orec")
                            nc.vector.reciprocal(out=rrec[:], in_=pr[0:HD, 0, :])
                            nc.vector.tensor_mul(
                                out=OT[hp][64 * h2:64 * (h2 + 1), :],
                                in0=po[h2][0:HD, :],
                                in1=rrec[:],
                            )

            # ---- phase 5: out projection;  phase 6: residual + ln_attn + ln2 ----
            with (
                tc.tile_pool(name="rot3", bufs=2) as rot3,
                tc.tile_pool(name="ps3", bufs=1, space="PSUM") as ps3,
            ):
                with nc.named_scope("outproj_ln"):
                    for t in range(TT):
                        y_sb = rot3.tile([128, C], F32, tag="ysb", bufs=2,
                                         name=f"ysb{t}")
                        for n2 in range(2):
                            py = ps3.tile([128, 512], F32, tag="y", bufs=3,
                                          name=f"py{t}{n2}")
                            for hp in range(HP):
                                nc.tensor.matmul(
                                    py[:],
                                    OT[hp][:, 128 * t:128 * (t + 1)],
                                    wo_sb[:, hp, 512 * n2:512 * (n2 + 1)],
                                    start=(hp == 0), stop=(hp == HP - 1),
                                )
                            nc.vector.tensor_tensor(
                                y_sb[:, 512 * n2:512 * (n2 + 1)], py[:],
                                outb_bc[:, 512 * n2:512 * (n2 + 1)], ALU.add,
                            )
                        mean, rstd = layer_norm_stats(y_sb, rot3)
                        nc.vector.tensor_scalar(out=y_sb[:], in0=y_sb[:],
                                                scalar1=mean, scalar2=rstd[:],
                                                op0=ALU.subtract, op1=ALU.mult)
                        nc.vector.tensor_mul(out=y_sb[:], in0=y_sb[:], in1=ga_bc[:])
                        nc.vector.tensor_add(out=y_sb[:], in0=y_sb[:], in1=ba_bc[:])
                        nc.vector.tensor_add(out=u_sb[t][:], in0=y_sb[:],
                                             in1=x_sb[t][:])
                        # LN2 -> h1 (bf16) -> transposed h1T
                        mean2, rstd2 = layer_norm_stats(u_sb[t], rot3)
                        h1 = rot3.tile([128, C], BF16, tag="h1")
                        nc.vector.tensor_scalar(out=h1[:], in0=u_sb[t][:],
                                                scalar1=mean2, scalar2=rstd2[:],
                                                op0=ALU.subtract, op1=ALU.mult)
                        for ct in range(CT):
                            ptr = ps3.tile([128, 128], BF16, tag="tr", bufs=3)
                            nc.tensor.transpose(
                                ptr[:], h1[:, 128 * ct:128 * (ct + 1)], ident[:]
                            )
                            nc.vector.tensor_copy(
                                out=h1T[ct][:, 128 * t:128 * (t + 1)], in_=ptr[:]
                            )

            # ---- phase 7: fc + gelu;  phase 8: proj + final residual ----
            with (
                tc.tile_pool(name="pd", bufs=1) as pd,
                tc.tile_pool(name="rot4", bufs=2) as rot4,
                tc.tile_pool(name="ps4", bufs=1, space="PSUM") as ps4,
            ):
                gT = [pd.tile([128, T], BF16, name=f"gT{m}") for m in range(MT_FC)]
                with nc.named_scope("fc"):
                    for mt in range(MT_FC):
                        wfc_t = rot4.tile([128, CT, 128], BF16, tag="wfc", bufs=6,
                                          name=f"wfc{mt}")
                        nc.sync.dma_start(
                            out=wfc_t[:],
                            in_=wfc_d[:, 128 * mt:128 * (mt + 1)].rearrange(
                                "(k p) m -> p k m", p=128
                            ),
                        )
                        pfc = ps4.tile([128, T], F32, tag="fc", bufs=3,
                                       name=f"pfc{mt}")
                        for kc in range(CT):
                            nc.tensor.matmul(
                                pfc[:], wfc_t[:, kc, :], h1T[kc][:],
                                start=(kc == 0), stop=(kc == CT - 1),
                            )
                        nc.scalar.activation(out=gT[mt][:], in_=pfc[:], func=AF.Gelu,
                                             bias=fcb_sb[:, mt:mt + 1])

                with nc.named_scope("proj"):
                    for n2 in range(2):
                        ppj = [
                            ps4.tile([128, 512], F32, tag=f"pj{t}", bufs=1,
                                     name=f"ppj{t}")
                            for t in range(TT)
                        ]
                        for mt in range(MT_FC):
                            wpj_t = rot4.tile([128, 512], BF16, tag="wpj", bufs=8,
                                              name=f"wpj{n2}_{mt}")
                            nc.sync.dma_start(
                                out=wpj_t[:],
                                in_=wpj_d[128 * mt:128 * (mt + 1),
                                          512 * n2:512 * (n2 + 1)],
                            )
                            for t in range(TT):
                                nc.tensor.matmul(
                                    ppj[t][:],
                                    gT[mt][:, 128 * t:128 * (t + 1)],
                                    wpj_t[:],
                                    start=(mt == 0), stop=(mt == MT_FC - 1),
                                )
                        for t in range(TT):
                            o1 = rot4.tile([128, 512], F32, tag="ofin",
                                           name=f"of{n2}_{t}")
                            nc.vector.tensor_tensor(
                                o1[:], ppj[t][:],
                                pjb_bc[:, 512 * n2:512 * (n2 + 1)], ALU.add,
                            )
                            nc.vector.tensor_add(
                                out=o1[:], in0=o1[:],
                                in1=u_sb[t][:, 512 * n2:512 * (n2 + 1)],
                            )
                            nc.sync.dma_start(
                                out=out_d[128 * t:128 * (t + 1),
                                          512 * n2:512 * (n2 + 1)],
                                in_=o1[:],
                            )

    nc.compile()
    return nc


def _host_prep(inp):
    f32 = np.float32
    ln1_g = np.asarray(inp["ln1_g"], f32)
    ln1_b = np.asarray(inp["ln1_b"], f32)
    ln2_g = np.asarray(inp["ln2_g"], f32)
    ln2_b = np.asarray(inp["ln2_b"], f32)
    in_w = np.asarray(inp["in_proj_w"], f32)
    in_b = np.asarray(inp["in_proj_b"], f32)
    fc_w = np.asarray(inp["fc_w"], f32)
    proj_w = np.asarray(inp["proj_w"], f32)
    ls1 = np.asarray(inp["ls1"], f32)
    ls2 = np.asarray(inp["ls2"], f32)

    w_qk = in_w[:2 * C]
    w_v = in_w[2 * C:]
    p = {}
    p["wqkT"] = np.ascontiguousarray((w_qk * ln1_g[None, :]).T).astype(BF_NP)
    p["wvT"] = np.ascontiguousarray((w_v * ln1_g[None, :]).T).astype(BF_NP)
    qkb = in_b[:2 * C] + ln1_b @ w_qk.T
    p["qkb"] = np.ascontiguousarray(qkb.reshape(16, 128).T).astype(f32)
    p["vb"] = (in_b[2 * C:] + ln1_b @ w_v.T).reshape(1, C).astype(f32)
    p["woT"] = np.ascontiguousarray(np.asarray(inp["out_w"], f32).T).astype(BF_NP)
    p["outb"] = np.asarray(inp["out_b"], f32).reshape(1, C)
    p["wfcT"] = np.ascontiguousarray((fc_w * ln2_g[None, :]).T).astype(BF_NP)
    fcb = np.asarray(inp["fc_b"], f32) + ln2_b @ fc_w.T
    p["fcb"] = np.ascontiguousarray(fcb.reshape(32, 128).T).astype(f32)
    p["wprojT"] = np.ascontiguousarray((proj_w * ls2[:, None]).T).astype(BF_NP)
    p["projb"] = (ls2 * np.asarray(inp["proj_b"], f32)).reshape(1, C)
    p["gattn"] = (ls1 * np.asarray(inp["ln_attn_g"], f32)).reshape(1, C)
    p["battn"] = (ls1 * np.asarray(inp["ln_attn_b"], f32)).reshape(1, C)

    lsc = np.exp(np.minimum(np.asarray(inp["logit_scale"], f32).reshape(H),
                            np.log(100.0)))
    onesq = np.zeros((128, 8, 128), f32)
    for mt in range(8):
        for blk in range(2):
            h = 2 * mt + blk
            onesq[64 * blk:64 * (blk + 1), mt,
                  64 * blk:64 * (blk + 1)] = 1.0 / lsc[h] ** 2
    p["onesq"] = np.ascontiguousarray(onesq.reshape(128, 1024))
    onesk = np.zeros((128, 128), f32)
    onesk[:64, :64] = 1.0
    onesk[64:, 64:] = 1.0
    p["onesk"] = onesk
    hs = np.asarray(inp["head_scale"], f32)
    invhs = np.zeros((1, C), f32)
    for h in range(H):
        invhs[0, HD * h:HD * (h + 1)] = 1.0 / hs[h]
    p["invhs"] = invhs
    return p


def kernel(**inputs) -> np.ndarray:
    global _NC_CACHE, LAST_EXEC_NS, LAST_RESULTS
    if _NC_CACHE is None:
        _NC_CACHE = _build()
    nc = _NC_CACHE

    p = _host_prep(inputs)
    x = np.asarray(inputs["x"], np.float32)

    in_maps = []
    for c in range(N_CORES):
        b, r = c // RANKS, c % RANKS
        m = dict(p)
        m["x"] = np.ascontiguousarray(x[b, T * r:T * (r + 1), :])
        in_maps.append(m)

    kwargs = {}
    if TRACE:
        import os
        os.makedirs(TRACE_DIR, exist_ok=True)
        kwargs = dict(trace=True, tmpdir=TRACE_DIR)
    res = run_bass_kernel_spmd(nc, in_maps, list(range(N_CORES)), **kwargs)
    LAST_EXEC_NS = res.exec_time_ns
    LAST_RESULTS = res
    out = np.zeros((B, L, C), np.float32)
    for c in range(N_CORES):
        b, r = c // RANKS, c % RANKS
        out[b, T * r:T * (r + 1), :] = res.results[c]["out"]
    return out
